# revision 31
# baseline (speedup 1.0000x reference)
"""Trainium2 Bass kernel for masked multi-head attention with a rope-like
positional transform (nn_Attention_43937515438607) — v7.

Architecture (per core, 2 batches):
  - Q,K projected in TRANSPOSED [d, tok] layout (lhsT = W chunk, rhs = x^T);
    V in natural layout with a ones-column for the softmax denominator.
  - rope: bias added in the PSUM->SBUF staging copy (tensor_scalar with a
    per-partition bias AP); pair-swap via one PE matmul against a constant
    128x128 pair-permutation; two fp16 multiplies + add on VectorE.
  - scores: two K=64 matmuls per kv tile into disjoint PE row groups
    (auto tile_position from base partitions 0/64) -> run CONCURRENTLY,
    issued ONE kv-tile AHEAD of the exp pipeline so the Activation engine
    (the ~156us exp floor) never waits on PE.
  - softmax: exp on ScalarE straight out of PSUM (no max subtraction,
    validated range), alpha in bf16; {0,1} mask applied as ONE broadcast
    tensor_tensor over both heads.
  - AV: V_ext (64 v rows + ones row) as stationary, alpha moving,
    accumulated over kv tiles; output normalized after a PE transpose
    (OT staged in bf16).
  - benchmark loop (niter>1) runs TWO unrolled iterations per For_i body
    with ping-ponged x/w/rope input buffer sets: the next iteration's
    input DMAs have WAR deps >= half an iteration old and stream during
    the current iteration's compute, hiding the ~26us input-DMA fill
    that a single-iteration body pays serially at every back-edge (the
    staggered-reset stage protocol allows almost no cross-back-edge
    overlap). Each iteration's PROLOGUE (first unit's Q/K projection +
    half of V(b0)) is fed into the PREVIOUS iteration's last attention
    unit, whose feed list is otherwise empty — erasing the pipeline
    refill seam (measured: 252us -> 205us with this + the DMA overlap).
  - y output DMAs issue from ScalarE with a 2-slot lag (never from SP,
    whose in-order queue must reach next-iteration input DMAs early);
    V/OT staging copies on ScalarE (DVE is the HW-critical engine).

Measured per-op HW costs that shaped this: exp[128,1024] 1218ns;
S-pair 225ns (concurrent); AV mm 222ns; DVE TT/copies per the
58/120+FD cost model (HW DVE is the bottleneck engine, per ablation);
GpSimd shares SBUF ports with DVE (no help for elementwise).
"""

import sys

try:
    import concourse  # noqa: F401
except ImportError:  # pragma: no cover
    sys.path.insert(0, "/opt/trn_rl_repo")

import numpy as np
import ml_dtypes

from concourse import bass, mybir, tile, bacc
from concourse.bass_utils import run_bass_kernel_spmd
from concourse.masks import make_identity

B, T, C = 16, 1024, 512
NH = 8
D = C // NH
TP = float((2.0 * D) ** 0.5)
N_CORES = 8
BPC = B // N_CORES            # batches per core = 2
TOK = BPC * T                 # tokens per core  = 2048
NTT = TOK // 128              # token tiles per core = 16
NTB = T // 128                # token tiles per batch = 8
NHP = NH // 2                 # head pairs = 4
QC = 512                      # q chunk (PSUM bank) per attention unit
NQC = T // QC                 # q chunks per batch = 2
NG = TOK // 512               # 512-token groups per core = 4
VW = 66 * NH + 32             # V_ext row width = 560

F32 = mybir.dt.float32
F32R = mybir.dt.float32r
F16 = mybir.dt.float16
BF16 = mybir.dt.bfloat16
MULT = mybir.AluOpType.mult
ADD = mybir.AluOpType.add

# how many of the 16 V-staging / 16 OT-staging copies run on ScalarE
# (the rest run on DVE); DVE is the HW-critical engine, so most go to Act.
V_ON_ACT = 16
OT_ON_ACT = 8
MASK_ON = True       # ablation knob (timing experiments only)
EXP_ON = True        # ablation knob (timing experiments only)


def build_nc(niter=1):
    nc = bacc.Bacc("TRN2", target_bir_lowering=False, debug=False)

    xT_d = nc.dram_tensor("xT", [C, TOK], F16, kind="ExternalInput")
    wT_d = nc.dram_tensor("wT", [C, 3 * C], F16, kind="ExternalInput")
    brow_d = nc.dram_tensor("brow_v", [1, C], F16, kind="ExternalInput")
    ones_d = nc.dram_tensor("ones_row", [1, 128], F16, kind="ExternalInput")
    swp_d = nc.dram_tensor("swp", [128, 128], F16, kind="ExternalInput")
    bqk_d = nc.dram_tensor("bias_qk", [128, 8], F32, kind="ExternalInput")
    rope_d = nc.dram_tensor("ropeT", [4, 128, T], F16, kind="ExternalInput")
    nmT_d = nc.dram_tensor("nmT", [BPC, T, T], BF16, kind="ExternalInput")
    y_d = nc.dram_tensor("y", [TOK, C], F32, kind="ExternalOutput")

    NBUF = 1 if niter == 1 else 2

    with tile.TileContext(nc) as tc:
        import contextlib
        trips, rem = divmod(niter, 2)
        loop_cm = (tc.For_i(0, trips, 1, staggered_reset=True,
                            hint_engines=(mybir.EngineType.PE,
                                          mybir.EngineType.DVE,
                                          mybir.EngineType.Activation,
                                          mybir.EngineType.SP))
                   if trips >= 1 and niter > 1 else contextlib.nullcontext())
        ctx = contextlib.ExitStack()
        with ctx:
            persist = ctx.enter_context(tc.tile_pool(name="persist", bufs=1))
            V_sb = persist.tile([128, NTT, VW], BF16)
            QT = [persist.tile([128, NHP, T], F16, tag=f"QT{b}", name=f"QT{b}")
                  for b in range(BPC)]
            KT = [persist.tile([128, NHP, T], F16, tag=f"KT{b}", name=f"KT{b}")
                  for b in range(BPC)]
            OT = [persist.tile([96, NH, T], BF16, tag=f"OT{b}", name=f"OT{b}")
                  for b in range(BPC)]
            mT = [persist.tile([128, NTB, T], BF16, tag=f"mT{b}",
                               name=f"mT{b}") for b in range(BPC)]
            bsets = [dict(
                xg=persist.tile([128, NG, 4, 512], F16,
                                tag=f"xg{s}", name=f"xg{s}"),
                wt=persist.tile([128, 4, 3 * C], F16, tag=f"wt{s}",
                                name=f"wt{s}"),
                rp=persist.tile([128, 4, T], F16, tag=f"rp{s}",
                                name=f"rp{s}"),
                swp=persist.tile([128, 128], F16, tag=f"swp{s}",
                                 name=f"swp{s}"),
                bqk=persist.tile([128, 8], F32, tag=f"bqk{s}",
                                 name=f"bqk{s}"),
                ones1=persist.tile([1, 128], F16, tag=f"ones{s}",
                                   name=f"ones{s}"),
                brow=persist.tile([1, C], F16, tag=f"brow{s}",
                                  name=f"brow{s}"),
            ) for s in range(NBUF)]
            id_tmp = persist.tile([128, 128], F32)
            id_bf = persist.tile([128, 128], BF16)

            # constant setup — input-independent, hoisted OUT of the loop.
            make_identity(nc, id_tmp[:])
            nc.vector.tensor_copy(id_bf[:], id_tmp[:])
            nc.gpsimd.memset(V_sb[:], 0.0)
            nc.vector.memset(V_sb[:, :, 64::66], 1.0)

            pp = ctx.enter_context(tc.tile_pool(name="pp", bufs=2, space="PSUM"))
            s_ps = ctx.enter_context(tc.tile_pool(name="s_ps", bufs=2, space="PSUM"))
            o_ps = ctx.enter_context(tc.tile_pool(name="o_ps", bufs=1, space="PSUM"))
            qsb_pool = ctx.enter_context(tc.tile_pool(name="qsb", bufs=2))
            t_pool = ctx.enter_context(tc.tile_pool(name="tpl", bufs=1))
            alpha_pool = ctx.enter_context(tc.tile_pool(name="alpha", bufs=5))
            fin_sb = ctx.enter_context(tc.tile_pool(name="fin_sb", bufs=3))

            vcnt = [0]
            ocnt = [0]

            def dma_mask(b, kg):
                nc.sync.dma_start(
                    mT[b][:, kg * 2:(kg + 1) * 2, :],
                    nmT_d[b][kg * 256:(kg + 1) * 256, :].rearrange(
                        "(kt p) q -> p kt q", p=128))

            def dma_pset(P):
                # x + weights + rope + consts for one ping-ponged buffer
                # set: WAR deps are >= half an iteration old, so these
                # stream during the current iteration's compute.
                for g in range(NG):
                    nc.sync.dma_start(
                        P["xg"][:, g],
                        xT_d[:, g * 512:(g + 1) * 512].rearrange(
                            "(ko p) t -> p ko t", p=128))
                for ko in range(4):
                    nc.sync.dma_start(P["wt"][:, ko, :],
                                      wT_d[ko * 128:(ko + 1) * 128, :])
                nc.sync.dma_start(P["rp"][:], rope_d.rearrange("f p t -> p f t"))
                nc.sync.dma_start(P["ones1"][:], ones_d[:])
                nc.sync.dma_start(P["brow"][:], brow_d[:])
                nc.sync.dma_start(P["swp"][:], swp_d[:])
                nc.sync.dma_start(P["bqk"][:], bqk_d[:])

            def v_chunk(P, b, g01, t):
                g = 2 * b + g01
                tt = 4 * g + t
                ps = pp.tile([128, 512], F32, tag="pp", name="vps")
                for ko in range(4):
                    nc.tensor.matmul(
                        ps[:], P["xg"][:, g, ko, t * 128:(t + 1) * 128],
                        P["wt"][:, ko, 2 * C:3 * C], start=(ko == 0),
                        stop=False)
                nc.tensor.matmul(ps[:], P["ones1"][:], P["brow"][:],
                                 start=False, stop=True)
                vdst = V_sb[:, tt, :528].rearrange(
                    "p (h e) -> p h e", h=NH)[:, :, :D]
                on_act = vcnt[0] % 16 < V_ON_ACT
                vcnt[0] += 1
                if on_act:
                    nc.scalar.copy(
                        vdst, ps[:].rearrange("p (h d) -> p h d", h=NH))
                else:
                    nc.vector.tensor_copy(
                        vdst, ps[:].rearrange("p (h d) -> p h d", h=NH))

            def qk_chunk(P, b, hp, fc, g01):
                col0 = fc * C + hp * 128
                dstt = (QT if fc == 0 else KT)[b]
                g = 2 * b + g01
                tsl = slice(g01 * 512, (g01 + 1) * 512)
                ps = pp.tile([128, 512], F32, tag="pp", name="qkps")
                for ko in range(4):
                    nc.tensor.matmul(
                        ps[:], P["wt"][:, ko, col0:col0 + 128],
                        P["xg"][:, g, ko, :], start=(ko == 0), stop=(ko == 3))
                qsb = qsb_pool.tile([128, 512], F16, tag="qsb")
                nc.vector.tensor_scalar(
                    qsb[:], ps[:], P["bqk"][:, 4 * fc + hp:4 * fc + hp + 1],
                    None, ADD)
                sw = pp.tile([128, 512], F32, tag="pp", name="swps")
                nc.tensor.matmul(sw[:], P["swp"][:], qsb[:],
                                 start=True, stop=True)
                t1 = t_pool.tile([128, 512], F16, tag="t1")
                nc.vector.tensor_tensor(t1[:], qsb[:],
                                        P["rp"][:, 2 * fc, tsl], MULT)
                t2 = t_pool.tile([128, 512], F16, tag="t2")
                nc.vector.tensor_tensor(t2[:], sw[:],
                                        P["rp"][:, 2 * fc + 1, tsl], MULT)
                nc.vector.tensor_tensor(dstt[:, hp, tsl], t1[:], t2[:],
                                        ADD)

            def prologue_chunks(s2):
                # first unit's Q/K projection + first half of V(b0),
                # bound to buffer set s2 — fed into the PREVIOUS
                # iteration's last attention unit (whose feed list is
                # otherwise empty, with PE half-idle there).
                P2 = bsets[s2]
                return ([lambda fc=fc, g01=g01: qk_chunk(P2, 0, 0, fc, g01)
                         for fc in range(2) for g01 in range(2)]
                        + [lambda t=t: v_chunk(P2, 0, 0, t)
                           for t in range(4)])

            def emit_body(s, nxt, tail):
                P = bsets[s]

                # masks: b0 frees ~50% into the previous iteration, b1 at
                # its end; SP blocks on each in turn, always well before
                # the consumer needs the data.
                for kg in range(4):
                    dma_mask(0, kg)
                for kg in range(4):
                    dma_mask(1, kg)

                pend_ydma = []

                def fin_chunk(b, half, qt):
                    out_sb = fin_sb.tile([128, C // 2], F32, tag="out")
                    fp = pp.tile([128, 4 * 96], BF16, tag="pp", name="fin")
                    for hh in range(4):
                        h = half * 4 + hh
                        nc.tensor.matmul(
                            fp[:, hh * 96:(hh + 1) * 96],
                            OT[b][:, h, qt * 128:(qt + 1) * 128],
                            id_bf[0:96, 0:96], is_transpose=True)
                    rc = fin_sb.tile([128, 4], F32, tag="rc")
                    nc.vector.reciprocal(rc[:], fp[:, 64::96])
                    nc.vector.tensor_tensor(
                        out_sb[:].rearrange("p (h d) -> p h d", h=4),
                        fp[:].rearrange("p (h e) -> p h e", e=96)[:, :, :D],
                        rc[:][:, :, None].to_broadcast([128, 4, D]), MULT)
                    row = b * T + qt * 128
                    pend_ydma.append((y_d[row:row + 128,
                                          half * 256:(half + 1) * 256],
                                      out_sb))

                def flush_ydma(force=False):
                    # Fed fin y DMAs issue from SP (idle mid-body; its
                    # in-order queue is past this body's input DMAs by
                    # then) with a >=3-slot lag; the epilogue fins (which
                    # would block SP across the For_i back-edge) issue
                    # from ScalarE instead (force=True).
                    if pend_ydma and (force or len(pend_ydma) >= 3):
                        dst, sb = pend_ydma.pop(0)
                        if force:
                            nc.scalar.dma_start(dst, sb[:])
                        else:
                            nc.sync.dma_start(dst, sb[:])

                def attention(b, hp, feed):
                    hA, hB = 2 * hp, 2 * hp + 1
                    for qc in range(NQC):
                        qsl = slice(qc * QC, (qc + 1) * QC)
                        oo = o_ps.tile([96, 2 * QC], F32, tag="oo")
                        oA, oB = oo[:, 0:QC], oo[:, QC:2 * QC]

                        def issue_scores(kt):
                            sp = s_ps.tile([128, 2 * QC], F32, tag="s")
                            nc.tensor.matmul(
                                sp[:, 0:QC],
                                KT[b][0:64, hp, kt * 128:(kt + 1) * 128],
                                QT[b][0:64, hp, qsl], start=True, stop=True)
                            nc.tensor.matmul(
                                sp[:, QC:2 * QC],
                                KT[b][64:128, hp, kt * 128:(kt + 1) * 128],
                                QT[b][64:128, hp, qsl], start=True, stop=True)
                            return sp

                        def emit_av(al, kt):
                            vbase = b * NTB + kt
                            nc.tensor.matmul(
                                oA, V_sb[:, vbase, hA * 66:hA * 66 + 96],
                                al[:, 0:QC],
                                start=(kt == 0), stop=(kt == NTB - 1))
                            nc.tensor.matmul(
                                oB, V_sb[:, vbase, hB * 66:hB * 66 + 96],
                                al[:, QC:2 * QC],
                                start=(kt == 0), stop=(kt == NTB - 1))

                        pend = []
                        sp_next = issue_scores(0)
                        for kt in range(NTB):
                            sp = sp_next
                            al = alpha_pool.tile([128, 2 * QC], BF16, tag="al")
                            if EXP_ON:
                                nc.scalar.activation(
                                    al[:], sp[:],
                                    mybir.ActivationFunctionType.Exp,
                                    scale=1.0 / TP)
                            else:
                                nc.scalar.copy(al[:], sp[:])
                            if kt + 1 < NTB:
                                sp_next = issue_scores(kt + 1)
                            if MASK_ON:
                                msl = mT[b][:, kt, qsl]
                                nc.vector.tensor_tensor(
                                    al[:].rearrange("p (h q) -> p h q", h=2),
                                    al[:].rearrange("p (h q) -> p h q", h=2),
                                    msl[:, None, :].to_broadcast(
                                        [128, 2, QC]), MULT)
                            pend.append((al, kt))
                            if len(pend) > 3:
                                emit_av(*pend.pop(0))
                            flush_ydma()
                            if feed:
                                feed.pop(0)()
                        for p_ in pend:
                            emit_av(*p_)
                        on_act = ocnt[0] % 16 < OT_ON_ACT
                        ocnt[0] += 1
                        ot_copy = (nc.scalar.copy if on_act
                                   else nc.vector.tensor_copy)
                        ot_copy(
                            OT[b][:, hA:hB + 1, qsl], oo[:].rearrange(
                                "p (h q) -> p h q", h=2))

                def qk_chunks(b, hp):
                    return [lambda b=b, hp=hp, fc=fc, g01=g01:
                            qk_chunk(P, b, hp, fc, g01)
                            for fc in range(2) for g01 in range(2)]

                def v_chunks(b):
                    return [lambda b=b, g01=g01, t=t: v_chunk(P, b, g01, t)
                            for g01 in range(2) for t in range(4)]

                def fin_chunks(b, half):
                    return [lambda b=b, half=half, qt=qt: fin_chunk(b, half, qt)
                            for qt in range(NTB)]

                # this body's prologue (qk(0,0) + v(0)[:4]) already ran in
                # the previous body's tail (or pre-loop). per-unit feeders:
                # future work drips into the attention kt-loop. qk chunks
                # front-loaded so wt/xg/rope have no consumers in the last
                # ~45% of the iteration.
                attention(0, 0, v_chunks(0)[4:] + qk_chunks(0, 1))
                attention(0, 1, qk_chunks(0, 2) + qk_chunks(0, 3))
                attention(0, 2, v_chunks(1) + qk_chunks(1, 0)
                          + fin_chunks(0, 0)[:4])
                attention(0, 3, qk_chunks(1, 1) + qk_chunks(1, 2)
                          + fin_chunks(0, 0)[4:])
                attention(1, 0, qk_chunks(1, 3) + fin_chunks(0, 1))
                attention(1, 1, [])
                if tail:
                    dma_pset(bsets[nxt])
                attention(1, 2, fin_chunks(1, 0))
                attention(1, 3, list(tail))
                for f in fin_chunks(1, 1):
                    f()
                    flush_ydma()
                while pend_ydma:
                    flush_ydma(force=True)

            # pre-loop: initial input DMA + first body's prologue (runs
            # once; in-loop iterations get their prologue from the
            # previous body's tail).
            dma_pset(bsets[0])
            for f in prologue_chunks(0):
                f()

            if trips >= 1 and niter > 1:
                with loop_cm:
                    for u in range(NBUF):
                        nxt = (u + 1) % NBUF
                        emit_body(u, nxt, prologue_chunks(nxt))
                if rem:
                    # odd iteration count: one more body outside the loop;
                    # its prologue/pset came from the loop's last tail.
                    emit_body(0, 1 % NBUF, [])
            else:
                emit_body(0, 0, [])

    nc.compile()
    return nc


_NC_CACHE = None


def _get_nc():
    global _NC_CACHE
    if _NC_CACHE is None:
        _NC_CACHE = build_nc()
    return _NC_CACHE


def prep_inputs(x, pe0, pe1, pe2, mask, W_qkv, b_qkv):
    """Host-side layout prep + per-core sharding. Returns list of in_maps."""
    x = np.asarray(x, dtype=np.float32)
    pe0 = np.asarray(pe0, dtype=np.float32).reshape(T, D)
    pe1 = np.asarray(pe1, dtype=np.float32).reshape(T, D)
    pe2 = np.asarray(pe2, dtype=np.float32).reshape(T, D)
    mask = np.asarray(mask).astype(bool).reshape(B, T, T)
    W_qkv = np.asarray(W_qkv, dtype=np.float32)
    b_qkv = np.asarray(b_qkv, dtype=np.float32)

    wT = np.ascontiguousarray(W_qkv.T).astype(np.float16)   # [C, 3C]
    brow_v = np.ascontiguousarray(
        b_qkv[None, 2 * C:3 * C]).astype(np.float16)        # [1, C]
    ones_row = np.ones((1, 128), dtype=np.float16)
    swp = np.kron(np.eye(64, dtype=np.float32),
                  np.array([[0, 1], [1, 0]], np.float32)
                  ).astype(np.float16)                      # [128,128] pair swap
    bias_qk = np.ascontiguousarray(
        b_qkv[:2 * C].reshape(8, 128).T)                    # [128, 8]

    # rope tables in [d, t] layout, duplicated across the two heads of a
    # partition chunk. Bq/Bk carry the rotate_half sign on the OUTPUT index:
    # out[2i] += -in[2i+1]*pe1, out[2i+1] += +in[2i]*pe1.
    sign = np.ones(D, dtype=np.float32)
    sign[0::2] = -1.0
    Aq = (pe0 * pe2).T                                      # [D, T]
    Bq = (pe1 * pe2).T * sign[:, None]
    Ak = (pe0 / pe2).T
    Bk = (pe1 / pe2).T * sign[:, None]
    ropeT = np.stack(
        [np.tile(t, (2, 1)) for t in (Aq, Bq, Ak, Bk)]
    ).astype(np.float16)                                    # [4, 128, T]

    notmask = (~mask).astype(ml_dtypes.bfloat16)            # [B,T,T] {0,1}
    in_maps = []
    for c in range(N_CORES):
        bs = slice(c * BPC, (c + 1) * BPC)
        xc = np.ascontiguousarray(
            x[bs].reshape(TOK, C).T).astype(np.float16)     # [C, TOK]
        nmT = np.ascontiguousarray(
            notmask[bs].transpose(0, 2, 1))                 # [BPC, T(kv), T(q)]
        in_maps.append(dict(
            xT=xc, wT=wT, brow_v=brow_v, ones_row=ones_row,
            swp=swp, bias_qk=bias_qk, ropeT=ropeT, nmT=nmT,
        ))
    return in_maps


def assemble_output(results):
    out = np.empty((B, T, C), dtype=np.float32)
    for c in range(N_CORES):
        out[c * BPC:(c + 1) * BPC] = results[c]["y"].reshape(BPC, T, C)
    return out


def kernel(x, pe0, pe1, pe2, mask, W_qkv, b_qkv):
    nc = _get_nc()
    in_maps = prep_inputs(x, pe0, pe1, pe2, mask, W_qkv, b_qkv)
    res = run_bass_kernel_spmd(nc, in_maps, core_ids=list(range(N_CORES)))
    return assemble_output(res.results)


# revision 33
# speedup vs baseline: 1.0826x; 1.0826x over previous
"""Trainium2 Bass kernel for masked multi-head attention with a rope-like
positional transform (nn_Attention_43937515438607) — v7.

Architecture (per core, 2 batches):
  - Q,K projected in TRANSPOSED [d, tok] layout (lhsT = W chunk, rhs = x^T);
    V in natural layout with a ones-column for the softmax denominator.
  - rope: bias added in the PSUM->SBUF staging copy (tensor_scalar with a
    per-partition bias AP); pair-swap via one PE matmul against a constant
    128x128 pair-permutation; two fp16 multiplies + add on VectorE.
  - scores: two K=64 matmuls per kv tile into disjoint PE row groups
    (auto tile_position from base partitions 0/64) -> run CONCURRENTLY,
    issued ONE kv-tile AHEAD of the exp pipeline so the Activation engine
    (the ~156us exp floor) never waits on PE.
  - softmax: exp on ScalarE straight out of PSUM (no max subtraction,
    validated range), alpha in bf16; {0,1} mask applied as ONE broadcast
    tensor_tensor over both heads.
  - AV: V_ext (64 v rows + ones row) as stationary, alpha moving,
    accumulated over kv tiles; output normalized after a PE transpose
    (OT staged in bf16).
  - benchmark loop (niter>1) runs TWO unrolled iterations per For_i body
    with ping-ponged x/w/rope input buffer sets: the next iteration's
    input DMAs have WAR deps >= half an iteration old and stream during
    the current iteration's compute, hiding the ~26us input-DMA fill
    that a single-iteration body pays serially at every back-edge (the
    staggered-reset stage protocol allows almost no cross-back-edge
    overlap). Each iteration's PROLOGUE (first unit's Q/K projection +
    half of V(b0)) is fed into the PREVIOUS iteration's last attention
    unit, whose feed list is otherwise empty — erasing the pipeline
    refill seam (measured: 252us -> 205us with this + the DMA overlap).
  - y output DMAs issue from ScalarE with a 2-slot lag (never from SP,
    whose in-order queue must reach next-iteration input DMAs early);
    V/OT staging copies on ScalarE (DVE is the HW-critical engine).

Measured per-op HW costs that shaped this: exp[128,1024] 1218ns;
S-pair 225ns (concurrent); AV mm 222ns; DVE TT/copies per the
58/120+FD cost model (HW DVE is the bottleneck engine, per ablation);
GpSimd shares SBUF ports with DVE (no help for elementwise).
"""

import sys

try:
    import concourse  # noqa: F401
except ImportError:  # pragma: no cover
    sys.path.insert(0, "/opt/trn_rl_repo")

import numpy as np
import ml_dtypes

from concourse import bass, mybir, tile, bacc
from concourse.bass_utils import run_bass_kernel_spmd
from concourse.masks import make_identity

B, T, C = 16, 1024, 512
NH = 8
D = C // NH
TP = float((2.0 * D) ** 0.5)
N_CORES = 8
BPC = B // N_CORES            # batches per core = 2
TOK = BPC * T                 # tokens per core  = 2048
NTT = TOK // 128              # token tiles per core = 16
NTB = T // 128                # token tiles per batch = 8
NHP = NH // 2                 # head pairs = 4
QC = 512                      # q chunk (PSUM bank) per attention unit
NQC = T // QC                 # q chunks per batch = 2
NG = TOK // 512               # 512-token groups per core = 4
VW = 66 * NH + 32             # V_ext row width = 560

F32 = mybir.dt.float32
F32R = mybir.dt.float32r
F16 = mybir.dt.float16
BF16 = mybir.dt.bfloat16
MULT = mybir.AluOpType.mult
ADD = mybir.AluOpType.add

# how many of the 16 V-staging / 16 OT-staging copies run on ScalarE
# (the rest run on DVE); DVE is the HW-critical engine, so most go to Act.
V_ON_ACT = 16
OT_ON_ACT = 16
MASK_ON = True       # ablation knob (timing experiments only)
EXP_ON = True        # ablation knob (timing experiments only)


def build_nc(niter=1):
    nc = bacc.Bacc("TRN2", target_bir_lowering=False, debug=False)

    xT_d = nc.dram_tensor("xT", [C, TOK], F16, kind="ExternalInput")
    wT_d = nc.dram_tensor("wT", [C, 3 * C], F16, kind="ExternalInput")
    brow_d = nc.dram_tensor("brow_v", [1, C], F16, kind="ExternalInput")
    ones_d = nc.dram_tensor("ones_row", [1, 128], F16, kind="ExternalInput")
    swp_d = nc.dram_tensor("swp", [128, 128], F16, kind="ExternalInput")
    bqk_d = nc.dram_tensor("bias_qk", [128, 8], F32, kind="ExternalInput")
    rope_d = nc.dram_tensor("ropeT", [4, 128, T], F16, kind="ExternalInput")
    nmT_d = nc.dram_tensor("nmT", [BPC, T, T], BF16, kind="ExternalInput")
    y_d = nc.dram_tensor("y", [TOK, C], F32, kind="ExternalOutput")

    NBUF = 1 if niter == 1 else 2

    with tile.TileContext(nc) as tc:
        import contextlib
        trips, rem = divmod(niter, 2)
        loop_cm = (tc.For_i(0, trips, 1, staggered_reset=True,
                            hint_engines=(mybir.EngineType.PE,
                                          mybir.EngineType.DVE,
                                          mybir.EngineType.Activation,
                                          mybir.EngineType.SP))
                   if trips >= 1 and niter > 1 else contextlib.nullcontext())
        ctx = contextlib.ExitStack()
        with ctx:
            persist = ctx.enter_context(tc.tile_pool(name="persist", bufs=1))
            V_sb = persist.tile([128, NTT, VW], BF16)
            QT = [persist.tile([128, NHP, T], F16, tag=f"QT{b}", name=f"QT{b}")
                  for b in range(BPC)]
            KT = [persist.tile([128, NHP, T], F16, tag=f"KT{b}", name=f"KT{b}")
                  for b in range(BPC)]
            OT = [persist.tile([96, NH, T], BF16, tag=f"OT{b}", name=f"OT{b}")
                  for b in range(BPC)]
            mT = [persist.tile([128, NTB, T], BF16, tag=f"mT{b}",
                               name=f"mT{b}") for b in range(BPC)]
            bsets = [dict(
                xg=persist.tile([128, NG, 4, 512], F16,
                                tag=f"xg{s}", name=f"xg{s}"),
                wt=persist.tile([128, 4, 3 * C], F16, tag=f"wt{s}",
                                name=f"wt{s}"),
                rp=persist.tile([128, 4, T], F16, tag=f"rp{s}",
                                name=f"rp{s}"),
                swp=persist.tile([128, 128], F16, tag=f"swp{s}",
                                 name=f"swp{s}"),
                bqk=persist.tile([128, 8], F32, tag=f"bqk{s}",
                                 name=f"bqk{s}"),
                ones1=persist.tile([1, 128], F16, tag=f"ones{s}",
                                   name=f"ones{s}"),
                brow=persist.tile([1, C], F16, tag=f"brow{s}",
                                  name=f"brow{s}"),
            ) for s in range(NBUF)]
            id_tmp = persist.tile([128, 128], F32)
            id_bf = persist.tile([128, 128], BF16)

            # constant setup — input-independent, hoisted OUT of the loop.
            make_identity(nc, id_tmp[:])
            nc.vector.tensor_copy(id_bf[:], id_tmp[:])
            nc.gpsimd.memset(V_sb[:], 0.0)
            nc.vector.memset(V_sb[:, :, 64::66], 1.0)

            pp = ctx.enter_context(tc.tile_pool(name="pp", bufs=2, space="PSUM"))
            s_ps = ctx.enter_context(tc.tile_pool(name="s_ps", bufs=2, space="PSUM"))
            o_ps = ctx.enter_context(tc.tile_pool(name="o_ps", bufs=1, space="PSUM"))
            qsb_pool = ctx.enter_context(tc.tile_pool(name="qsb", bufs=2))
            t_pool = ctx.enter_context(tc.tile_pool(name="tpl", bufs=1))
            alpha_pool = ctx.enter_context(tc.tile_pool(name="alpha", bufs=5))
            fin_sb = ctx.enter_context(tc.tile_pool(name="fin_sb", bufs=3))

            vcnt = [0]
            ocnt = [0]

            def dma_mask(b, kg):
                nc.sync.dma_start(
                    mT[b][:, kg * 2:(kg + 1) * 2, :],
                    nmT_d[b][kg * 256:(kg + 1) * 256, :].rearrange(
                        "(kt p) q -> p kt q", p=128))

            def dma_pset(P):
                # x + weights + rope + consts for one ping-ponged buffer
                # set: WAR deps are >= half an iteration old, so these
                # stream during the current iteration's compute.
                for g in range(NG):
                    nc.sync.dma_start(
                        P["xg"][:, g],
                        xT_d[:, g * 512:(g + 1) * 512].rearrange(
                            "(ko p) t -> p ko t", p=128))
                for ko in range(4):
                    nc.sync.dma_start(P["wt"][:, ko, :],
                                      wT_d[ko * 128:(ko + 1) * 128, :])
                nc.sync.dma_start(P["rp"][:], rope_d.rearrange("f p t -> p f t"))
                nc.sync.dma_start(P["ones1"][:], ones_d[:])
                nc.sync.dma_start(P["brow"][:], brow_d[:])
                nc.sync.dma_start(P["swp"][:], swp_d[:])
                nc.sync.dma_start(P["bqk"][:], bqk_d[:])

            def v_chunk(P, b, g01, t):
                g = 2 * b + g01
                tt = 4 * g + t
                ps = pp.tile([128, 512], F32, tag="pp", name="vps")
                for ko in range(4):
                    nc.tensor.matmul(
                        ps[:], P["xg"][:, g, ko, t * 128:(t + 1) * 128],
                        P["wt"][:, ko, 2 * C:3 * C], start=(ko == 0),
                        stop=False)
                nc.tensor.matmul(ps[:], P["ones1"][:], P["brow"][:],
                                 start=False, stop=True)
                vdst = V_sb[:, tt, :528].rearrange(
                    "p (h e) -> p h e", h=NH)[:, :, :D]
                on_act = vcnt[0] % 16 < V_ON_ACT
                vcnt[0] += 1
                if on_act:
                    nc.scalar.copy(
                        vdst, ps[:].rearrange("p (h d) -> p h d", h=NH))
                else:
                    nc.vector.tensor_copy(
                        vdst, ps[:].rearrange("p (h d) -> p h d", h=NH))

            def qk_chunk(P, b, hp, fc, g01):
                col0 = fc * C + hp * 128
                dstt = (QT if fc == 0 else KT)[b]
                g = 2 * b + g01
                tsl = slice(g01 * 512, (g01 + 1) * 512)
                ps = pp.tile([128, 512], F32, tag="pp", name="qkps")
                for ko in range(4):
                    nc.tensor.matmul(
                        ps[:], P["wt"][:, ko, col0:col0 + 128],
                        P["xg"][:, g, ko, :], start=(ko == 0), stop=(ko == 3))
                qsb = qsb_pool.tile([128, 512], F16, tag="qsb")
                nc.scalar.add(qsb[:], ps[:],
                              P["bqk"][:, 4 * fc + hp:4 * fc + hp + 1])
                sw = pp.tile([128, 512], F32, tag="pp", name="swps")
                nc.tensor.matmul(sw[:], P["swp"][:], qsb[:],
                                 start=True, stop=True)
                t1 = t_pool.tile([128, 512], F16, tag="t1")
                nc.vector.tensor_tensor(t1[:], qsb[:],
                                        P["rp"][:, 2 * fc, tsl], MULT)
                t2 = t_pool.tile([128, 512], F16, tag="t2")
                nc.vector.tensor_tensor(t2[:], sw[:],
                                        P["rp"][:, 2 * fc + 1, tsl], MULT)
                nc.vector.tensor_tensor(dstt[:, hp, tsl], t1[:], t2[:],
                                        ADD)

            def prologue_chunks(s2):
                # first unit's Q/K projection + first half of V(b0),
                # bound to buffer set s2 — fed into the PREVIOUS
                # iteration's last attention unit (whose feed list is
                # otherwise empty, with PE half-idle there).
                P2 = bsets[s2]
                return ([lambda fc=fc, g01=g01: qk_chunk(P2, 0, 0, fc, g01)
                         for fc in range(2) for g01 in range(2)]
                        + [lambda t=t: v_chunk(P2, 0, 0, t)
                           for t in range(4)])

            def emit_body(s, nxt, tail):
                P = bsets[s]

                # masks: b0 frees ~50% into the previous iteration, b1 at
                # its end; SP blocks on each in turn, always well before
                # the consumer needs the data.
                for kg in range(4):
                    dma_mask(0, kg)
                for kg in range(4):
                    dma_mask(1, kg)

                pend_ydma = []

                def fin_chunk(b, half, qt):
                    out_sb = fin_sb.tile([128, C // 2], F32, tag="out")
                    fp = pp.tile([128, 4 * 96], BF16, tag="pp", name="fin")
                    for hh in range(4):
                        h = half * 4 + hh
                        nc.tensor.matmul(
                            fp[:, hh * 96:(hh + 1) * 96],
                            OT[b][:, h, qt * 128:(qt + 1) * 128],
                            id_bf[0:96, 0:96], is_transpose=True)
                    rc = fin_sb.tile([128, 4], F32, tag="rc")
                    nc.vector.reciprocal(rc[:], fp[:, 64::96])
                    nc.vector.tensor_tensor(
                        out_sb[:].rearrange("p (h d) -> p h d", h=4),
                        fp[:].rearrange("p (h e) -> p h e", e=96)[:, :, :D],
                        rc[:][:, :, None].to_broadcast([128, 4, D]), MULT)
                    row = b * T + qt * 128
                    pend_ydma.append((y_d[row:row + 128,
                                          half * 256:(half + 1) * 256],
                                      out_sb))

                def flush_ydma(force=False):
                    # Fed fin y DMAs issue from SP (idle mid-body; its
                    # in-order queue is past this body's input DMAs by
                    # then) with a >=3-slot lag; the epilogue fins (which
                    # would block SP across the For_i back-edge) issue
                    # from ScalarE instead (force=True).
                    if pend_ydma and (force or len(pend_ydma) >= 3):
                        dst, sb = pend_ydma.pop(0)
                        if force:
                            nc.scalar.dma_start(dst, sb[:])
                        else:
                            nc.sync.dma_start(dst, sb[:])

                def attention(b, hp, feed):
                    hA, hB = 2 * hp, 2 * hp + 1
                    for qc in range(NQC):
                        qsl = slice(qc * QC, (qc + 1) * QC)
                        oo = o_ps.tile([96, 2 * QC], F32, tag="oo")
                        oA, oB = oo[:, 0:QC], oo[:, QC:2 * QC]

                        def issue_scores(kt):
                            sp = s_ps.tile([128, 2 * QC], F32, tag="s")
                            nc.tensor.matmul(
                                sp[:, 0:QC],
                                KT[b][0:64, hp, kt * 128:(kt + 1) * 128],
                                QT[b][0:64, hp, qsl], start=True, stop=True)
                            nc.tensor.matmul(
                                sp[:, QC:2 * QC],
                                KT[b][64:128, hp, kt * 128:(kt + 1) * 128],
                                QT[b][64:128, hp, qsl], start=True, stop=True)
                            return sp

                        def emit_av(al, kt):
                            vbase = b * NTB + kt
                            nc.tensor.matmul(
                                oA, V_sb[:, vbase, hA * 66:hA * 66 + 96],
                                al[:, 0:QC],
                                start=(kt == 0), stop=(kt == NTB - 1))
                            nc.tensor.matmul(
                                oB, V_sb[:, vbase, hB * 66:hB * 66 + 96],
                                al[:, QC:2 * QC],
                                start=(kt == 0), stop=(kt == NTB - 1))

                        pend = []
                        sp_next = issue_scores(0)
                        for kt in range(NTB):
                            sp = sp_next
                            al = alpha_pool.tile([128, 2 * QC], BF16, tag="al")
                            if EXP_ON:
                                nc.scalar.activation(
                                    al[:], sp[:],
                                    mybir.ActivationFunctionType.Exp,
                                    scale=1.0 / TP)
                            else:
                                nc.scalar.copy(al[:], sp[:])
                            if kt + 1 < NTB:
                                sp_next = issue_scores(kt + 1)
                            if MASK_ON:
                                msl = mT[b][:, kt, qsl]
                                nc.vector.tensor_tensor(
                                    al[:].rearrange("p (h q) -> p h q", h=2),
                                    al[:].rearrange("p (h q) -> p h q", h=2),
                                    msl[:, None, :].to_broadcast(
                                        [128, 2, QC]), MULT)
                            pend.append((al, kt))
                            if len(pend) > 3:
                                emit_av(*pend.pop(0))
                            flush_ydma()
                            if feed:
                                feed.pop(0)()
                        for p_ in pend:
                            emit_av(*p_)
                        on_act = ocnt[0] % 16 < OT_ON_ACT
                        ocnt[0] += 1
                        ot_copy = (nc.scalar.copy if on_act
                                   else nc.vector.tensor_copy)
                        ot_copy(
                            OT[b][:, hA:hB + 1, qsl], oo[:].rearrange(
                                "p (h q) -> p h q", h=2))

                def qk_chunks(b, hp):
                    return [lambda b=b, hp=hp, fc=fc, g01=g01:
                            qk_chunk(P, b, hp, fc, g01)
                            for fc in range(2) for g01 in range(2)]

                def v_chunks(b):
                    return [lambda b=b, g01=g01, t=t: v_chunk(P, b, g01, t)
                            for g01 in range(2) for t in range(4)]

                def fin_chunks(b, half):
                    return [lambda b=b, half=half, qt=qt: fin_chunk(b, half, qt)
                            for qt in range(NTB)]

                # this body's prologue (qk(0,0) + v(0)[:4]) already ran in
                # the previous body's tail (or pre-loop). per-unit feeders:
                # future work drips into the attention kt-loop. qk chunks
                # front-loaded so wt/xg/rope have no consumers in the last
                # ~45% of the iteration.
                attention(0, 0, v_chunks(0)[4:] + qk_chunks(0, 1))
                attention(0, 1, qk_chunks(0, 2) + qk_chunks(0, 3))
                attention(0, 2, v_chunks(1) + qk_chunks(1, 0)
                          + fin_chunks(0, 0)[:4])
                attention(0, 3, qk_chunks(1, 1) + qk_chunks(1, 2)
                          + fin_chunks(0, 0)[4:])
                attention(1, 0, qk_chunks(1, 3) + fin_chunks(0, 1))
                attention(1, 1, [])
                if tail:
                    dma_pset(bsets[nxt])
                attention(1, 2, fin_chunks(1, 0))
                attention(1, 3, list(tail))
                for f in fin_chunks(1, 1):
                    f()
                    flush_ydma()
                while pend_ydma:
                    flush_ydma(force=True)

            # pre-loop: initial input DMA + first body's prologue (runs
            # once; in-loop iterations get their prologue from the
            # previous body's tail).
            dma_pset(bsets[0])
            for f in prologue_chunks(0):
                f()

            if trips >= 1 and niter > 1:
                with loop_cm:
                    for u in range(NBUF):
                        nxt = (u + 1) % NBUF
                        emit_body(u, nxt, prologue_chunks(nxt))
                if rem:
                    # odd iteration count: one more body outside the loop;
                    # its prologue/pset came from the loop's last tail.
                    emit_body(0, 1 % NBUF, [])
            else:
                emit_body(0, 0, [])

    nc.compile()
    return nc


_NC_CACHE = None


def _get_nc():
    global _NC_CACHE
    if _NC_CACHE is None:
        _NC_CACHE = build_nc()
    return _NC_CACHE


def prep_inputs(x, pe0, pe1, pe2, mask, W_qkv, b_qkv):
    """Host-side layout prep + per-core sharding. Returns list of in_maps."""
    x = np.asarray(x, dtype=np.float32)
    pe0 = np.asarray(pe0, dtype=np.float32).reshape(T, D)
    pe1 = np.asarray(pe1, dtype=np.float32).reshape(T, D)
    pe2 = np.asarray(pe2, dtype=np.float32).reshape(T, D)
    mask = np.asarray(mask).astype(bool).reshape(B, T, T)
    W_qkv = np.asarray(W_qkv, dtype=np.float32)
    b_qkv = np.asarray(b_qkv, dtype=np.float32)

    wT = np.ascontiguousarray(W_qkv.T).astype(np.float16)   # [C, 3C]
    brow_v = np.ascontiguousarray(
        b_qkv[None, 2 * C:3 * C]).astype(np.float16)        # [1, C]
    ones_row = np.ones((1, 128), dtype=np.float16)
    swp = np.kron(np.eye(64, dtype=np.float32),
                  np.array([[0, 1], [1, 0]], np.float32)
                  ).astype(np.float16)                      # [128,128] pair swap
    bias_qk = np.ascontiguousarray(
        b_qkv[:2 * C].reshape(8, 128).T)                    # [128, 8]

    # rope tables in [d, t] layout, duplicated across the two heads of a
    # partition chunk. Bq/Bk carry the rotate_half sign on the OUTPUT index:
    # out[2i] += -in[2i+1]*pe1, out[2i+1] += +in[2i]*pe1.
    sign = np.ones(D, dtype=np.float32)
    sign[0::2] = -1.0
    Aq = (pe0 * pe2).T                                      # [D, T]
    Bq = (pe1 * pe2).T * sign[:, None]
    Ak = (pe0 / pe2).T
    Bk = (pe1 / pe2).T * sign[:, None]
    ropeT = np.stack(
        [np.tile(t, (2, 1)) for t in (Aq, Bq, Ak, Bk)]
    ).astype(np.float16)                                    # [4, 128, T]

    notmask = (~mask).astype(ml_dtypes.bfloat16)            # [B,T,T] {0,1}
    in_maps = []
    for c in range(N_CORES):
        bs = slice(c * BPC, (c + 1) * BPC)
        xc = np.ascontiguousarray(
            x[bs].reshape(TOK, C).T).astype(np.float16)     # [C, TOK]
        nmT = np.ascontiguousarray(
            notmask[bs].transpose(0, 2, 1))                 # [BPC, T(kv), T(q)]
        in_maps.append(dict(
            xT=xc, wT=wT, brow_v=brow_v, ones_row=ones_row,
            swp=swp, bias_qk=bias_qk, ropeT=ropeT, nmT=nmT,
        ))
    return in_maps


def assemble_output(results):
    out = np.empty((B, T, C), dtype=np.float32)
    for c in range(N_CORES):
        out[c * BPC:(c + 1) * BPC] = results[c]["y"].reshape(BPC, T, C)
    return out


def kernel(x, pe0, pe1, pe2, mask, W_qkv, b_qkv):
    nc = _get_nc()
    in_maps = prep_inputs(x, pe0, pe1, pe2, mask, W_qkv, b_qkv)
    res = run_bass_kernel_spmd(nc, in_maps, core_ids=list(range(N_CORES)))
    return assemble_output(res.results)
